# revision 28
# baseline (speedup 1.0000x reference)
"""Trainium2 Bass kernel: row-wise Dempster-Shafer combination of two
Dirichlet opinions (C = 21 classes, N = 2097152 rows).

The reference computes, per row:
    S_k = sum(alpha_k);  b_k = (alpha_k - 1)/S_k;  u_k = C/S_k
    K = sum(b0)*sum(b1) - dot(b0, b1);  denom = 1 - K
    b = (b0*b1 + b0*u1 + b1*u0)/denom;  u = u0*u1/denom
    alpha_out = b*(C/u) + 1

Algebraically `denom` cancels out of alpha_out entirely and the whole map
collapses to the elementwise closed form

    alpha_out = (alpha1 + C-1) * (alpha2 + C-1) / C - (C-1)

(max rel err vs the fp32 reference ~3e-6 — pure rounding).  So the kernel
is a pure streaming elementwise pipeline: rows are sharded across the 8
NeuronCores (data parallel, no communication), each core streams its
contiguous 22 MB block of both inputs through SBUF in 8 chunks, applies
three fused elementwise ops (1 ACT + 2 DVE), and writes the result back.
Memory-bound by design: ~66 MB of HBM traffic per core.
"""

import numpy as np

import concourse.bacc as bacc
import concourse.bass as bass
import concourse.tile as tile
from concourse import mybir
from concourse.bass_utils import run_bass_kernel_spmd

N_CORES = 8
N_ROWS = 2097152
C = 21
PER = N_ROWS // N_CORES          # 262144 rows per core
ELEMS = PER * C                  # 5505024 f32 elements per tensor per core
P = 128                          # SBUF partitions
FREE = ELEMS // P                # 43008 contiguous f32 per partition
F = 10752                        # main chunk width: 5.5 MB DMAs (best measured)

_nc_cache = {}


def _build(repeats=1, F=F, bufs=2, rings="sp", mode="full", ramp=True):
    """Build the Bass program. `repeats` re-runs the whole streaming pipeline
    N times inside one NEFF — used by the test harness to measure pure device
    time as a slope between two repeat counts (cancels dispatch overhead).
    rings="sp": loads SP-HWDGE, stores ACT-HWDGE.
    rings="split": a1 loads SP, a2 loads ACT, stores gpsimd SWDGE.
    mode: "full" = real kernel; "copy"/"loadonly" = BW-probe variants
    (wrong results, bench-only).
    ramp: lead the FIRST pass with small chunks so compute/stores start
    ~45 us earlier — shortens the pipeline-fill edge of a single
    execution without touching steady state (pool slots are sized to the
    largest tag tile, so SBUF cost is unchanged)."""
    key = (repeats, F, bufs, rings, mode, ramp)
    if key in _nc_cache:
        return _nc_cache[key]
    assert FREE % F == 0
    main_sizes = [F] * (FREE // F)
    ramp_sizes = [F // 4, F // 4, F // 2] + [F] * ((FREE - F) // F)
    assert sum(ramp_sizes) == FREE == sum(main_sizes)
    # Bacc (not raw Bass): its compile() runs generate_event_semaphores,
    # which legalizes multi-sem dependencies to the HW limit of one sync
    # wait per instruction by inserting EventSemaphore instructions.
    nc = bacc.Bacc(None)
    a1 = nc.dram_tensor("alpha1", [P, FREE], mybir.dt.float32, kind="ExternalInput")
    a2 = nc.dram_tensor("alpha2", [P, FREE], mybir.dt.float32, kind="ExternalInput")
    out = nc.dram_tensor("out", [P, FREE], mybir.dt.float32, kind="ExternalOutput")

    with tile.TileContext(nc) as tc:
        with (
            tc.tile_pool(name="t1", bufs=bufs) as pool1,
            tc.tile_pool(name="t2", bufs=bufs) as pool2,
        ):
            schedule = []
            for r in range(repeats):
                sizes = ramp_sizes if (ramp and r == 0) else main_sizes
                off = 0
                for sz in sizes:
                    schedule.append((off, sz, len(schedule) % 2))
                    off += sz
            for off, sz, parity in schedule:
                sl = slice(off, off + sz)
                t1 = pool1.tile([P, sz], mybir.dt.float32, name="t1", tag="t1")
                t2 = (pool2.tile([P, sz], mybir.dt.float32, name="t2", tag="t2")
                      if mode != "load1" else None)
                if rings == "split":
                    a2_loader, storer = nc.scalar, nc.gpsimd
                elif rings == "mix":
                    # loads split across both HWDGE rings; stores alternate
                    # so each ring carries ~3 MB per chunk
                    a2_loader = nc.scalar
                    storer = nc.sync if parity == 0 else nc.scalar
                else:
                    a2_loader, storer = nc.sync, nc.scalar
                nc.sync.dma_start(out=t1[:], in_=a1[:, sl])
                if mode != "load1":
                    a2_loader.dma_start(out=t2[:], in_=a2[:, sl])
                if mode == "full":
                    # All compute on DVE: the NEFF encoding allows only ONE
                    # sync-wait per instruction, and a single engine makes
                    # every in-engine dependency ride the same semaphore
                    # (mergeable), so each op waits on at most one sem.
                    # t1 = a1 + 20             (tensor_scalar, 2x mode)
                    nc.vector.tensor_scalar_add(t1[:], t1[:], float(C - 1))
                    # t2 = (a2 + 20) * (1/21)  (fused 2-scalar-op, 2x mode)
                    nc.vector.tensor_scalar(
                        t2[:], t2[:], float(C - 1), float(1.0 / C),
                        op0=mybir.AluOpType.add, op1=mybir.AluOpType.mult,
                    )
                    # t1 = t1 * t2             (tensor_tensor, 1x mode)
                    nc.vector.tensor_mul(t1[:], t1[:], t2[:])
                    # t1 = t1 - 20             (tensor_scalar, 2x mode)
                    nc.vector.tensor_scalar_add(t1[:], t1[:], float(-(C - 1)))
                if mode not in ("loadonly", "load1"):
                    # Store off the SP ring: stores wait on compute, and on
                    # the SP ring that wait would block the in-order
                    # sequencer from issuing later loads.
                    storer.dma_start(out=out[:, sl], in_=t1[:])
    # Bacc defers register allocation etc. to compile(), which finalize()
    # runs; the bass2jax exec path serializes without finalizing.
    nc.finalize()
    _nc_cache[key] = nc
    return nc


def _run(alpha1, alpha2, trace=False, repeats=1, **kwargs):
    nc = _build(repeats)
    alpha1 = np.ascontiguousarray(np.asarray(alpha1, dtype=np.float32))
    alpha2 = np.ascontiguousarray(np.asarray(alpha2, dtype=np.float32))
    in_maps = []
    for c in range(N_CORES):
        blk = slice(c * PER, (c + 1) * PER)
        in_maps.append({
            "alpha1": alpha1[blk].reshape(P, FREE),
            "alpha2": alpha2[blk].reshape(P, FREE),
        })
    res = run_bass_kernel_spmd(nc, in_maps, list(range(N_CORES)), trace=trace, **kwargs)
    full = np.empty((N_ROWS, C), dtype=np.float32)
    for c in range(N_CORES):
        full[c * PER:(c + 1) * PER] = res.results[c]["out"].reshape(PER, C)
    return full, res


def kernel(alpha1, alpha2):
    return _run(alpha1, alpha2)[0]
